# revision 1
# baseline (speedup 1.0000x reference)
import sys

sys.path.insert(0, "/opt/trn_rl_repo")
import numpy as np
import ml_dtypes
import concourse.bass as bass
import concourse.tile as tile
from concourse import mybir, masks
from concourse.bass_utils import run_bass_kernel_spmd


# CoreV3 codegen allows only ONE sync wait on a sync-engine drain; the stock
# final drain waits on every live sem at once. Emit one drain per nonzero
# clock proc instead (each gets a single sem wait).
def _split_drain_and_barrier(self, tick_clock, wait_clock):
    from concourse.vector_clock import ScopedClock, VectorClock

    nc = self.nc
    gc = tick_clock.global_clock
    n = len(gc)
    emitted = False
    for p in range(n):
        t = gc[p]
        if t == 0:
            continue
        vec = [0] * n
        vec[p] = t
        d = nc.sync.drain()
        wait_clock.add_sem_waits(d.ins, ScopedClock({None: VectorClock(vec)}))
        emitted = True
    if not emitted:
        d = nc.sync.drain()
        wait_clock.add_sem_waits(d.ins, ScopedClock({None: gc}))
    nc.all_engine_barrier()
    assert self.sems is not None
    popped = nc._tile_sem_poison_stack.pop()
    assert popped is self._sem_poison
    nc.clear_and_free_semaphores(list(self.sems.allocated().values()))
    nc.all_engine_barrier()


tile.TileContext._drain_and_barrier = _split_drain_and_barrier

NCORES = 8
T, R, E, B = 4, 64, 1024, 128
IN = R + 2 * E  # 2112
EC = E // NCORES  # 128 entity cols per core
FCH = E // 128  # 8 f-chunks of 128
NCH = (IN + 127) // 128  # 17 input chunks
INP = NCH * 128  # 2176 padded input dim
G4 = 4 * R  # 256 gate width

f32 = mybir.dt.float32
bf16 = mybir.dt.bfloat16
AF = mybir.ActivationFunctionType
ALU = mybir.AluOpType
AX = mybir.AxisListType


def build_program():
    nc = bass.Bass()
    # counter sem for DVE wait absorbers; alloc BEFORE TileContext so the id
    # is not one the tile pools free and reuse mid-program
    cap_sem = nc.alloc_semaphore("cap_absorb")
    kbt_d = nc.declare_dram_parameter("kbt", [128, FCH * R * EC], bf16, isOutput=False)
    mem0_d = nc.declare_dram_parameter("mem0", [B, E], f32, isOutput=False)
    tail_d = nc.declare_dram_parameter("tail", [B, EC], f32, isOutput=False)
    xtp_d = nc.declare_dram_parameter("xtp", [128, NCH * B], bf16, isOutput=False)
    w0_d = nc.declare_dram_parameter("w0", [128, NCH * G4], bf16, isOutput=False)
    whh_d = nc.declare_dram_parameter("whh", [R, T * G4], f32, isOutput=False)
    wih_d = nc.declare_dram_parameter("wih", [R, (T - 1) * G4], f32, isOutput=False)
    bias_d = nc.declare_dram_parameter("bias", [1, T * G4], f32, isOutput=False)
    out_d = nc.declare_dram_parameter("out", [B, 1], f32, isOutput=True)

    with tile.TileContext(nc) as tc:
        with tc.tile_pool(name="ps", bufs=8, space="PSUM") as ps, \
             tc.tile_pool(name="dram", bufs=8, space="DRAM") as dram:
            _frees = []

            def mktile(shape, dtype, **kw):
                t, f = tc.tile(shape, dtype, **kw)
                _frees.append(f)
                return t

            # ---- load constants / weights ----
            kbt = mktile([128, FCH * R * EC], bf16, name="kbt_sb")
            engs = [nc.gpsimd, nc.scalar, nc.sync]
            for fc in range(FCH):
                sl = slice(fc * R * EC, (fc + 1) * R * EC)
                engs[fc % 3].dma_start(kbt[:, sl], kbt_d[:, sl])

            mf0 = mktile([B, E], f32, name="mf0")
            nc.gpsimd.dma_start(mf0[:], mem0_d[:])
            tailb = mktile([B, EC], f32, name="tail_sb")
            nc.gpsimd.dma_start(tailb[:], tail_d[:])
            xtp = mktile([128, NCH * B], bf16, name="xtp_sb")
            nc.sync.dma_start(xtp[:], xtp_d[:])
            w0 = mktile([128, NCH * G4], bf16, name="w0_sb")
            nc.scalar.dma_start(w0[:], w0_d[:])
            whh = mktile([R, T * G4], f32, name="whh_sb")
            nc.gpsimd.dma_start(whh[:], whh_d[:])
            wih = mktile([R, (T - 1) * G4], f32, name="wih_sb")
            nc.gpsimd.dma_start(wih[:], wih_d[:])
            biasr = mktile([1, T * G4], f32, name="bias_sb")
            nc.gpsimd.dma_start(biasr[:], bias_d[:])
            ones = mktile([1, B], f32, name="ones_sb")
            nc.vector.memset(ones[:], 1.0)
            ident = mktile([128, 128], f32, name="ident_sb")
            masks.make_identity(nc, ident[:])

            # ---- LSTM: pre0 = x @ Wih0.T + bias0 (same for all t) ----
            pre0 = mktile([B, G4], f32, name="pre0_sb")
            p0 = ps.tile([B, G4], f32, name='p0', tag='bank')
            for q in range(NCH):
                nc.tensor.matmul(
                    p0[:], xtp[:, q * B:(q + 1) * B], w0[:, q * G4:(q + 1) * G4],
                    start=(q == 0), stop=False,
                )
            nc.tensor.matmul(p0[:], ones[:], biasr[:, 0:G4], start=False, stop=True)
            nc.scalar.copy(pre0[:], p0[:])

            # ---- LSTM stack ----
            hcur = [mktile([B, R], f32, name=f"h_{t}") for t in range(T)]
            hprv = [mktile([B, R], f32, name=f"hp_{t}") for t in range(T)]
            hTc = [mktile([R, B], f32, name=f"hT_{t}") for t in range(T)]
            hTp = [mktile([R, B], f32, name=f"hTp_{t}") for t in range(T)]
            ctile = mktile([B, R], f32, name="c_sb")
            itg = mktile([B, R], f32, name="itg_sb")
            sif = mktile([B, 2 * R], f32, name="sif_sb")
            tg = mktile([B, R], f32, name="tg_sb")
            so = mktile([B, R], f32, name="so_sb")
            thc = mktile([B, R], f32, name="thc_sb")
            zsb = mktile([B, G4], f32, name="z_sb")

            for l in range(T):
                if l > 0:
                    hprv, hcur = hcur, hprv
                    hTp, hTc = hTc, hTp
                for t in range(T):
                    if l == 0:
                        if t == 0:
                            z = pre0
                        else:
                            pz = ps.tile([B, G4], f32, name='pz', tag='bank')
                            nc.tensor.matmul(pz[:], hTc[t - 1][:], whh[:, 0:G4],
                                             start=True, stop=True)
                            nc.vector.tensor_add(zsb[:], pre0[:], pz[:])
                            z = zsb
                    else:
                        pz = ps.tile([B, G4], f32, name='pz', tag='bank')
                        nc.tensor.matmul(pz[:], hTp[t][:],
                                         wih[:, (l - 1) * G4:l * G4],
                                         start=True, stop=False)
                        if t > 0:
                            nc.tensor.matmul(pz[:], hTc[t - 1][:],
                                             whh[:, l * G4:(l + 1) * G4],
                                             start=False, stop=False)
                        nc.tensor.matmul(pz[:], ones[:],
                                         biasr[:, l * G4:(l + 1) * G4],
                                         start=False, stop=True)
                        z = pz
                    nc.scalar.activation(sif[:], z[:, 0:2 * R], AF.Sigmoid)
                    nc.scalar.activation(tg[:], z[:, 2 * R:3 * R], AF.Tanh)
                    nc.scalar.activation(so[:], z[:, 3 * R:4 * R], AF.Sigmoid)
                    if t == 0:
                        nc.vector.tensor_mul(ctile[:], sif[:, 0:R], tg[:])
                    else:
                        nc.vector.tensor_mul(ctile[:], sif[:, R:2 * R], ctile[:])
                        nc.vector.tensor_mul(itg[:], sif[:, 0:R], tg[:])
                        nc.vector.tensor_add(ctile[:], ctile[:], itg[:])
                    nc.scalar.activation(thc[:], ctile[:], AF.Tanh)
                    nc.vector.tensor_mul(hcur[t][:], so[:], thc[:])
                    pt = ps.tile([R, B], f32, name='pt', tag='bank')
                    nc.tensor.transpose(pt[:], hcur[t][:], ident[:])
                    nc.scalar.copy(hTc[t][:], pt[:])

            hs = hcur  # final-layer hidden states [B, R] x T

            # ---- softmaxes ----
            negmax = mktile([B, 1], f32, name="negmax")
            ssum = mktile([B, 1], f32, name="ssum")
            rsum = mktile([B, 1], f32, name="rsum")
            exps = mktile([B, R], f32, name="exps")

            def softmax(dst, src, n):
                nc.vector.tensor_reduce(negmax[:], src, AX.X, ALU.max, negate=True)
                nc.scalar.activation(exps[:, 0:n], src, AF.Exp,
                                     bias=negmax[:], accum_out=ssum[:])
                nc.vector.reciprocal(rsum[:], ssum[:])
                nc.scalar.mul(dst, exps[:, 0:n], rsum[:])

            hsm = [mktile([B, R], f32, name=f"hsm{t}") for t in range(T)]
            h2 = [mktile([B, R], f32, name=f"h2_{t}") for t in range(T)]
            for t in range(T):
                softmax(hsm[t][:], hs[t][:], R)
            for t in range(T):
                softmax(h2[t][:], hsm[t][:], R)

            # ---- attention weights (all precomputable from hsm) ----
            attl = [mktile([B, 4], f32, name=f"attl{i}") for i in range(T)]
            att = [mktile([B, 4], f32, name=f"att{i}") for i in range(T)]
            tscr = mktile([B, R], f32, name="ttr_scr")
            for i in range(1, T):
                for k in range(i + 1):
                    # TTR lowers to a DVE InstISA this walrus build rejects;
                    # use mul + reduce instead
                    nc.vector.tensor_mul(tscr[:], hsm[k][:], hsm[i][:])
                    nc.vector.tensor_reduce(attl[i][:, k:k + 1], tscr[:],
                                            AX.X, ALU.add)
                softmax(att[i][:, 0:i + 1], attl[i][:, 0:i + 1], i + 1)

            # ---- memory loop ----
            mfs = [mf0] + [mktile([B, E], f32, name=f"mf{k}") for k in (1, 2, 3)]
            pa = mktile([B, E], f32, name="prev_a")
            pb = mktile([B, E], f32, name="prev_b")
            prevT = mktile([128, E], bf16, name="prevT_sb")
            acc = mktile([B, EC], f32, name="acc_sb")
            zcol = mktile([B, 1], f32, name="zc_sb")
            zsum = mktile([B, 1], f32, name="zsum_sb")
            osb = mktile([B, 1], f32, name="out_sb")
            fscr = mktile([B, EC], f32, name="fin_scr")

            ag_sh = [mktile([NCORES * B, EC], f32, space="DRAM",
                             addr_space="Shared", name=f"ag{i}")
                     for i in range(3)]
            zred = mktile([B, 1], f32, space="DRAM",
                              addr_space="Shared", name="zred")

            for i in range(T):
                # prev = sum_k att[i][:,k] * mem_k  (i=0: att == [1.0] exactly)
                if i == 0:
                    prev = mf0
                else:
                    pp = [pa, pb]
                    cur = None
                    for k in range(i + 1):
                        dst = pp[k % 2]
                        if k == 0:
                            nc.vector.scalar_tensor_tensor(
                                dst[:], mfs[0][:], att[i][:, 0:1], mfs[0][:],
                                ALU.mult, ALU.bypass)
                        else:
                            nc.vector.scalar_tensor_tensor(
                                dst[:], mfs[k][:], att[i][:, k:k + 1], cur[:],
                                ALU.mult, ALU.add)
                        cur = dst
                    prev = cur
                # prevT (bf16) via PE transposes
                for fc in range(FCH):
                    ptp = ps.tile([128, 128], f32, name='ptp', tag='bank')
                    nc.tensor.transpose(ptp[:], prev[:, fc * 128:(fc + 1) * 128],
                                        ident[:])
                    nc.scalar.copy(prevT[:, fc * 128:(fc + 1) * 128], ptp[:])
                # tmp[b, (r, e')] = sum_f prev[b, f] * kb[r, c*EC+e', f]
                # acc[b, e'] = sum_r h2[i][b, r] * tmp[b, (r, e')]
                first = True
                for half in range(2):
                    pts = [ps.tile([B, 512], f32, name=f'pmm{half}_{jj}', tag='bank') for jj in range(8)]
                    for fc in range(FCH):
                        for j in range(8):
                            rg = half * 8 + j
                            nc.tensor.matmul(
                                pts[j][:], prevT[:, fc * 128:(fc + 1) * 128],
                                kbt[:, fc * R * EC + rg * 512:
                                     fc * R * EC + (rg + 1) * 512],
                                start=(fc == 0), stop=(fc == FCH - 1))
                    for j in range(8):
                        rg = half * 8 + j
                        for rl in range(4):
                            r = rg * 4 + rl
                            src = pts[j][:, rl * 128:(rl + 1) * 128]
                            if first:
                                nc.vector.scalar_tensor_tensor(
                                    acc[:], src, h2[i][:, r:r + 1], acc[:],
                                    ALU.mult, ALU.bypass)
                                first = False
                            else:
                                nc.vector.scalar_tensor_tensor(
                                    acc[:], src, h2[i][:, r:r + 1], acc[:],
                                    ALU.mult, ALU.add)
                if i < 3:
                    bounce = dram.tile([B, EC], f32, name='bounce')
                    nc.gpsimd.dma_start(bounce[:], acc[:])
                    nc.gpsimd.collective_compute(
                        "AllGather", ALU.bypass,
                        replica_groups=[list(range(NCORES))],
                        ins=[bounce.opt()], outs=[ag_sh[i].opt()])
                    for src_c in range(NCORES):
                        nc.gpsimd.dma_start(
                            mfs[i + 1][:, src_c * EC:(src_c + 1) * EC],
                            ag_sh[i][src_c * B:(src_c + 1) * B, :])
                else:
                    nc.vector.tensor_mul(fscr[:], acc[:], tailb[:])
                    nc.vector.tensor_reduce(zcol[:], fscr[:], AX.X, ALU.add)
                    zb = dram.tile([B, 1], f32, name='zb')
                    nc.gpsimd.dma_start(zb[:], zcol[:])
                    nc.gpsimd.collective_compute(
                        "AllReduce", ALU.add,
                        replica_groups=[list(range(NCORES))],
                        ins=[zb.opt()], outs=[zred.opt()])
                    nc.gpsimd.dma_start(zsum[:], zred[:])
                    nc.scalar.activation(osb[:], zsum[:], AF.Sigmoid,
                                         bias=0.0, scale=-1.0)
                    nc.gpsimd.dma_start(out_d[:], osb[:])
            for f in reversed(_frees):
                f()
    # CoreV3 allows at most 1 sync wait per instruction (2 on EventSemaphore);
    # reuse the Bacc rust passes to split overloaded waits.
    from concourse.bacc import _bass_rust
    _bass_rust.move_matmul_waits_to_ldweights(nc.m)
    _cap_pe_waits(nc, cap_sem)
    return nc


_CAP_SKIP = ("InstDrain", "InstEventSemaphore",
             "InstCollectiveCompute", "InstUnconditionalBranch", "InstCall")


def _cap_pe_waits(nc, cap_sem):
    # CoreV3 engine command structs hold only 1 sync wait. PE/Activation get
    # excess waits moved onto same-engine EventSemaphore insts. DVE (and any
    # other engine) cannot carry event sems through lower_dve, so their waits
    # are absorbed by Activation-engine event sems that each inc a shared
    # counter; the instruction then waits counter >= running total.
    act_eng = nc.scalar.engine
    total = 0
    for fn in nc.m.functions:
        for bb in fn.blocks:
            snapshot = list(bb.instructions)
            edits = []
            for k, ins in enumerate(snapshot):
                if ins.__class__.__name__ in _CAP_SKIP:
                    continue
                eng = str(getattr(ins, "engine", "")).split(".")[-1]
                si = ins.sync_info
                if si is None or len(si.on_wait) <= 1:
                    continue
                waits = list(si.on_wait)
                evs = []
                if eng in ("PE", "Activation"):
                    ins.sync_info = mybir.SyncInfo(
                        on_wait=[waits[-1]], on_update=list(si.on_update))
                    for w in waits[:-1]:
                        ev = mybir.InstEventSemaphore(
                            name=nc.get_next_instruction_name())
                        ev.engine = ins.engine
                        ev.sync_info = mybir.SyncInfo(on_wait=[w], on_update=[])
                        nc.register_instruction(ev)
                        evs.append(ev)
                else:
                    for w in waits:
                        ev = mybir.InstEventSemaphore(
                            name=nc.get_next_instruction_name())
                        ev.engine = act_eng
                        ev.sync_info = mybir.SyncInfo(
                            on_wait=[w],
                            on_update=[mybir.SyncUpdate(
                                sync_type='semaphore', id=cap_sem.num,
                                ant_name=cap_sem.name,
                                update_mode='sem-inc', update_value=1)])
                        nc.register_instruction(ev)
                        evs.append(ev)
                        total += 1
                    ins.sync_info = mybir.SyncInfo(
                        on_wait=[mybir.SyncWait(
                            sync_type='semaphore', id=cap_sem.num,
                            ant_name=cap_sem.name,
                            wait_mode='sem-ge-imm', wait_value=total)],
                        on_update=list(si.on_update))
                # never split a Ldweights/Matmult pair
                kk = k
                while kk > 0 and snapshot[kk - 1].__class__.__name__ == "InstLdweights":
                    kk -= 1
                edits.append((kk, evs))
            edits.sort(key=lambda e: e[0])  # stable: equal kk keeps discovery order
            for k, evs in reversed(edits):
                for ev in reversed(evs):
                    bb.instructions.insert(k, ev)


def _prep_inputs(inputs):
    x = np.asarray(inputs["x"], np.float32)
    kb = np.asarray(inputs["kb"], np.float32)
    Wih0 = np.asarray(inputs["Wih0"], np.float32)
    Whh0 = np.asarray(inputs["Whh0"], np.float32)
    bih0 = np.asarray(inputs["bih0"], np.float32)
    bhh0 = np.asarray(inputs["bhh0"], np.float32)
    Wih = np.asarray(inputs["Wih"], np.float32)
    Whh = np.asarray(inputs["Whh"], np.float32)
    bih = np.asarray(inputs["bih"], np.float32)
    bhh = np.asarray(inputs["bhh"], np.float32)

    # kbt[c][f, fc*R*EC + r*EC + e'] = kb[r, c*EC+e', fc*128+f]
    kb5 = kb.reshape(R, NCORES, EC, FCH, 128)
    kbt_all = np.ascontiguousarray(
        kb5.transpose(1, 4, 3, 0, 2)).reshape(NCORES, 128, FCH * R * EC)
    kbt_all = kbt_all.astype(ml_dtypes.bfloat16)

    mem0 = np.ascontiguousarray(x[:, R:R + E])
    tails = [np.ascontiguousarray(x[:, R + E + c * EC:R + E + (c + 1) * EC])
             for c in range(NCORES)]

    # xtp[p, q*B + j] = x[j, q*128 + p] (zero-padded input dim)
    xT = np.zeros((INP, B), np.float32)
    xT[:IN] = x.T
    xtp = np.ascontiguousarray(
        xT.reshape(NCH, 128, B).transpose(1, 0, 2)).reshape(128, NCH * B)
    xtp = xtp.astype(ml_dtypes.bfloat16)

    # w0[p, q*G4 + g] = Wih0[g, q*128 + p] (zero-padded input dim)
    w0T = np.zeros((INP, G4), np.float32)
    w0T[:IN] = Wih0.T
    w0 = np.ascontiguousarray(
        w0T.reshape(NCH, 128, G4).transpose(1, 0, 2)).reshape(128, NCH * G4)
    w0 = w0.astype(ml_dtypes.bfloat16)

    whhT = np.concatenate([Whh0.T] + [Whh[l].T for l in range(T - 1)], axis=1)
    whhT = np.ascontiguousarray(whhT)
    wihT = np.ascontiguousarray(
        np.concatenate([Wih[l].T for l in range(T - 1)], axis=1))
    biasr = np.concatenate(
        [bih0 + bhh0] + [bih[l] + bhh[l] for l in range(T - 1)])[None, :]
    biasr = np.ascontiguousarray(biasr.astype(np.float32))

    in_maps = []
    for c in range(NCORES):
        in_maps.append({
            "kbt": kbt_all[c],
            "mem0": mem0,
            "tail": tails[c],
            "xtp": xtp,
            "w0": w0,
            "whh": whhT,
            "wih": wihT,
            "bias": biasr,
        })
    return in_maps


_CACHED = {}


def kernel(**inputs) -> np.ndarray:
    if "nc" not in _CACHED:
        _CACHED["nc"] = build_program()
    nc = _CACHED["nc"]
    in_maps = _prep_inputs(inputs)
    res = run_bass_kernel_spmd(nc, in_maps, list(range(NCORES)), trace=False)
    out = np.asarray(res.results[0]["out"], np.float32).reshape(B, 1)
    return out


if __name__ == "__main__":
    rng = np.random.default_rng(0)
    demo = {
        "x": rng.uniform(size=(B, IN)).astype(np.float32),
        "kb": (rng.uniform(size=(R, E, E)) * 0.01).astype(np.float32),
        "Wih0": (rng.standard_normal((G4, IN)) * 0.05).astype(np.float32),
        "Whh0": (rng.standard_normal((G4, R)) * 0.05).astype(np.float32),
        "bih0": np.zeros((G4,), np.float32),
        "bhh0": np.zeros((G4,), np.float32),
        "Wih": (rng.standard_normal((T - 1, G4, R)) * 0.05).astype(np.float32),
        "Whh": (rng.standard_normal((T - 1, G4, R)) * 0.05).astype(np.float32),
        "bih": np.zeros((T - 1, G4), np.float32),
        "bhh": np.zeros((T - 1, G4), np.float32),
    }
    print(kernel(**demo)[:4, 0])



# revision 2
# speedup vs baseline: 1.5798x; 1.5798x over previous
import sys

sys.path.insert(0, "/opt/trn_rl_repo")
import numpy as np
import ml_dtypes
import concourse.bass as bass
import concourse.tile as tile
from concourse import mybir, masks
from concourse.bass_utils import run_bass_kernel_spmd


# CoreV3 codegen allows only ONE sync wait on a sync-engine drain; the stock
# final drain waits on every live sem at once. Emit one drain per nonzero
# clock proc instead (each gets a single sem wait).
def _split_drain_and_barrier(self, tick_clock, wait_clock):
    from concourse.vector_clock import ScopedClock, VectorClock

    nc = self.nc
    gc = tick_clock.global_clock
    n = len(gc)
    emitted = False
    for p in range(n):
        t = gc[p]
        if t == 0:
            continue
        vec = [0] * n
        vec[p] = t
        d = nc.sync.drain()
        wait_clock.add_sem_waits(d.ins, ScopedClock({None: VectorClock(vec)}))
        emitted = True
    if not emitted:
        d = nc.sync.drain()
        wait_clock.add_sem_waits(d.ins, ScopedClock({None: gc}))
    nc.all_engine_barrier()
    assert self.sems is not None
    popped = nc._tile_sem_poison_stack.pop()
    assert popped is self._sem_poison
    nc.clear_and_free_semaphores(list(self.sems.allocated().values()))
    nc.all_engine_barrier()


tile.TileContext._drain_and_barrier = _split_drain_and_barrier

NCORES = 8
T, R, E, B = 4, 64, 1024, 128
IN = R + 2 * E  # 2112
EC = E // NCORES  # 128 entity cols per core
DCH = 4  # 256-wide f chunks for DoubleRow
NCH = (IN + 127) // 128  # 17 input chunks
INP = NCH * 128  # 2176 padded input dim
G4 = 4 * R  # 256 gate width
SC = 32.0  # fp8 kb scale; folded out via h2/SC

f32 = mybir.dt.float32
bf16 = mybir.dt.bfloat16
fp8 = mybir.dt.float8e4
AF = mybir.ActivationFunctionType
ALU = mybir.AluOpType
AX = mybir.AxisListType
PM = mybir.MatmulPerfMode


def build_program():
    nc = bass.Bass()
    # counter sem for DVE wait absorbers; alloc BEFORE TileContext so the id
    # is not one the tile pools free and reuse mid-program
    cap_sem = nc.alloc_semaphore("cap_absorb")
    kbt_d = nc.declare_dram_parameter("kbt", [128, 16 * DCH * 1024], fp8, isOutput=False)
    m0t_d = nc.declare_dram_parameter("m0t", [128, DCH * 2 * B], fp8, isOutput=False)
    m0s_d = nc.declare_dram_parameter("m0s", [B, EC], f32, isOutput=False)
    tail_d = nc.declare_dram_parameter("tail", [B, EC], f32, isOutput=False)
    xtp_d = nc.declare_dram_parameter("xtp", [128, NCH * B], bf16, isOutput=False)
    w0_d = nc.declare_dram_parameter("w0", [128, NCH * G4], bf16, isOutput=False)
    whh_d = nc.declare_dram_parameter("whh", [R, T * G4], bf16, isOutput=False)
    wih_d = nc.declare_dram_parameter("wih", [R, (T - 1) * G4], bf16, isOutput=False)
    bias_d = nc.declare_dram_parameter("bias", [128, T * 2], f32, isOutput=False)
    out_d = nc.declare_dram_parameter("out", [B, 1], f32, isOutput=True)

    with tile.TileContext(nc) as tc:
        with tc.tile_pool(name="ps", bufs=6, space="PSUM") as ps, \
             tc.tile_pool(name="psl", bufs=2, space="PSUM") as psl, \
             tc.tile_pool(name="dram", bufs=8, space="DRAM") as dram:
            _frees = []

            def mktile(shape, dtype, **kw):
                t, f = tc.tile(shape, dtype, **kw)
                _frees.append(f)
                return t

            # ---- small tensors first so the LSTM can start immediately ----
            # xtp/w0 in interleaved pieces so the pre0T matmul for chunk q
            # can start as soon as its piece lands
            xtp = mktile([128, NCH * B], bf16, name="xtp_sb")
            w0 = mktile([128, NCH * G4], bf16, name="w0_sb")
            _qcuts = [0, 5, 10, 14, NCH]
            for a, b in zip(_qcuts[:-1], _qcuts[1:]):
                nc.sync.dma_start(xtp[:, a * B:b * B], xtp_d[:, a * B:b * B])
                nc.sync.dma_start(w0[:, a * G4:b * G4], w0_d[:, a * G4:b * G4])
            whh = mktile([R, T * G4], bf16, name="whh_sb")
            nc.sync.dma_start(whh[:], whh_d[:])
            wih = mktile([R, (T - 1) * G4], bf16, name="wih_sb")
            nc.sync.dma_start(wih[:], wih_d[:])
            biasr = mktile([128, T * 2], f32, name="bias_sb")
            nc.sync.dma_start(biasr[:], bias_d[:])
            m0t = mktile([128, DCH * 2 * B], fp8, name="m0t_sb")
            nc.sync.dma_start(m0t[:], m0t_d[:])
            m0s = mktile([B, EC], f32, name="m0s_sb")
            nc.sync.dma_start(m0s[:], m0s_d[:])
            tailb = mktile([B, EC], f32, name="tail_sb")
            nc.sync.dma_start(tailb[:], tail_d[:])

            # ---- kb (fp8, DoubleRow layout) in 16 pieces ----
            kbt = mktile([128, 16 * DCH * 1024], fp8, name="kbt_sb")
            for pc in range(16):
                sl = slice(pc * DCH * 1024, (pc + 1) * DCH * 1024)
                nc.sync.dma_start(kbt[:, sl], kbt_d[:, sl])

            identf = mktile([128, 128], f32, name="identf_sb")
            masks.make_identity(nc, identf[:])
            identb = mktile([128, 128], bf16, name="identb_sb")
            nc.scalar.copy(identb[:], identf[:])


            # ---- pre0T = (x @ Wih0.T).T in two gate blocks, bf16 ----
            p0a = mktile([128, B], bf16, name="p0a_sb")
            p0b = mktile([128, B], bf16, name="p0b_sb")
            pp0 = psl.tile([128, B], f32, name="pp0", tag="bank")
            pp1 = psl.tile([128, B], f32, name="pp1", tag="bank")
            for q in range(NCH):
                nc.tensor.matmul(
                    pp0[:], w0[:, q * G4:q * G4 + 128], xtp[:, q * B:(q + 1) * B],
                    start=(q == 0), stop=(q == NCH - 1))
            for q in range(NCH):
                nc.tensor.matmul(
                    pp1[:], w0[:, q * G4 + 128:(q + 1) * G4], xtp[:, q * B:(q + 1) * B],
                    start=(q == 0), stop=(q == NCH - 1))
            nc.scalar.copy(p0a[:], pp0[:])
            nc.scalar.copy(p0b[:], pp1[:])

            # ---- LSTM stack, transposed state: hT [R, B] bf16 per (l, t) ----
            # cells are emitted in wavefront (l+t) order and the per-t head
            # post-processing (h transpose, softmaxes, attention) is emitted
            # as soon as the final-layer cell for that t exists, so h2[0] is
            # as early as possible in every engine queue.
            hT = [[mktile([R, B], bf16, name=f"hT_{l}_{t}") for t in range(T)]
                  for l in range(T)]
            cTs = [mktile([R, B], bf16, name=f"cT_{l}") for l in range(T)]

            def emit_cell(l, t):
                cT = cTs[l]
                    bcol1i = biasr[0:R, 2 * l:2 * l + 1]
                    bcol1f = biasr[R:128, 2 * l:2 * l + 1]
                    bcol2g = biasr[0:R, 2 * l + 1:2 * l + 2]
                    bcol2o = biasr[R:128, 2 * l + 1:2 * l + 2]
                    if l == 0 and t == 0:
                        zin1, zin2 = p0a, p0b
                    else:
                        z1 = psl.tile([128, B], f32, name=f"z1_{l}_{t}", tag="bank")
                        z2 = psl.tile([128, B], f32, name=f"z2_{l}_{t}", tag="bank")
                        if l == 0:
                            nc.tensor.matmul(z1[:], whh[:, 0:128], hT[0][t - 1][:],
                                             start=True, stop=False)
                            nc.tensor.matmul(z1[:], identb[:], p0a[:],
                                             start=False, stop=True)
                            nc.tensor.matmul(z2[:], whh[:, 128:256], hT[0][t - 1][:],
                                             start=True, stop=False)
                            nc.tensor.matmul(z2[:], identb[:], p0b[:],
                                             start=False, stop=True)
                        else:
                            wof = (l - 1) * G4
                            nc.tensor.matmul(z1[:], wih[:, wof:wof + 128],
                                             hT[l - 1][t][:],
                                             start=True, stop=(t == 0))
                            if t > 0:
                                nc.tensor.matmul(z1[:], whh[:, l * G4:l * G4 + 128],
                                                 hT[l][t - 1][:],
                                                 start=False, stop=True)
                            nc.tensor.matmul(z2[:], wih[:, wof + 128:wof + 256],
                                             hT[l - 1][t][:],
                                             start=True, stop=(t == 0))
                            if t > 0:
                                nc.tensor.matmul(z2[:],
                                                 whh[:, l * G4 + 128:(l + 1) * G4],
                                                 hT[l][t - 1][:],
                                                 start=False, stop=True)
                        zin1, zin2 = z1, z2
                    siT = mktile([R, B], bf16, name=f"si_{l}_{t}")
                    sfT = mktile([R, B], bf16, name=f"sf_{l}_{t}")
                    gtT = mktile([R, B], bf16, name=f"gt_{l}_{t}")
                    soT = mktile([R, B], bf16, name=f"so_{l}_{t}")
                    thT = mktile([R, B], bf16, name=f"th_{l}_{t}")
                    itg = mktile([R, B], bf16, name=f"itg_{l}_{t}")
                    nc.scalar.activation(siT[:], zin1[0:R], AF.Sigmoid,
                                         bias=bcol1i)
                    if t > 0:
                        nc.scalar.activation(sfT[:], zin1[R:128], AF.Sigmoid,
                                             bias=bcol1f)
                    nc.scalar.activation(gtT[:], zin2[0:R], AF.Tanh, bias=bcol2g)
                    nc.scalar.activation(soT[:], zin2[R:128], AF.Sigmoid,
                                         bias=bcol2o)
                    if t == 0:
                        nc.vector.tensor_mul(cT[:], siT[:], gtT[:])
                    else:
                        nc.vector.tensor_mul(cT[:], sfT[:], cT[:])
                        nc.vector.tensor_mul(itg[:], siT[:], gtT[:])
                        nc.vector.tensor_add(cT[:], cT[:], itg[:])
                    nc.scalar.activation(thT[:], cT[:], AF.Tanh)
                    nc.vector.tensor_mul(hT[l][t][:], soT[:], thT[:])

            # ---- softmaxes (private scratch per call, parallelizable) ----
            # every softmax input here is bounded in (-1, 1], so the usual
            # max-subtraction is unnecessary — exp() directly
            def softmax(dst, src, n, tag, fold_sc=False):
                ssum = mktile([B, 1], f32, name=f"ssm_{tag}")
                rsum = mktile([B, 1], f32, name=f"rsm_{tag}")
                exps = mktile([B, n], f32, name=f"exp_{tag}")
                nc.scalar.activation(exps[:], src, AF.Exp,
                                     bias=0.0, accum_out=ssum[:])
                if fold_sc:
                    ssc = mktile([B, 1], f32, name=f"ssc_{tag}")
                    nc.scalar.mul(ssc[:], ssum[:], SC)
                    nc.vector.reciprocal(rsum[:], ssc[:])
                else:
                    nc.vector.reciprocal(rsum[:], ssum[:])
                nc.scalar.mul(dst, exps[:], rsum[:])

            h = [mktile([B, R], f32, name=f"h_{t}") for t in range(T)]
            hsm = [mktile([B, R], f32, name=f"hsm{t}") for t in range(T)]
            h2 = [mktile([B, R], f32, name=f"h2_{t}") for t in range(T)]
            att = [None] + [mktile([B, 4], f32, name=f"att{i}") for i in range(1, T)]
            attl = [None] + [mktile([B, 4], f32, name=f"attl{i}") for i in range(1, T)]

            def emit_head(t):
                # transpose final-layer hT -> h, then both softmaxes and the
                # attention row for this t
                pt = psl.tile([B, R], bf16, name=f"pt_{t}", tag="bank")
                nc.tensor.transpose(pt[:], hT[T - 1][t][:], identb[0:R, 0:R])
                nc.scalar.copy(h[t][:], pt[:])
                softmax(hsm[t][:], h[t][:], R, f"a{t}")
                # h2 carries a 1/SC fold to cancel the fp8 kb scale
                softmax(h2[t][:], hsm[t][:], R, f"b{t}", fold_sc=True)
                if t >= 1:
                    for k in range(t + 1):
                        scr = mktile([B, R], f32, name=f"ad_{t}_{k}")
                        nc.vector.scalar_tensor_tensor(
                            scr[:], hsm[k][:], 1.0, hsm[t][:], ALU.bypass, ALU.mult,
                            accum_out=attl[t][:, k:k + 1])
                    softmax(att[t][:, 0:t + 1], attl[t][:, 0:t + 1], t + 1, f"t{t}")

            # ---- memory loop, entity-sharded; mem slices stay local ----
            # r-reduction split: rl 0 (and rl 1 of even groups) accumulate on
            # DVE via an STT chain; the rest are scaled by Act into sacc
            # (bf16) and summed by one packed DVE reduce, then one add
            # combines the partials. Matmuls run in rounds of <=6 psum banks
            # (the other 2 are the LSTM's own pool).
            ROUNDS = [list(range(0, 6)), list(range(6, 12)), list(range(12, 16))]

            def r_on_dve(rg, rl):
                return rl == 0 or (rl == 1 and rg % 4 != 3)

            act_k = {}
            for rg in range(16):
                for rl in range(4):
                    if not r_on_dve(rg, rl):
                        act_k[(rg, rl)] = len(act_k)
            NACT = len(act_k)  # 40
            NACTH = NACT // 2  # act k's split evenly across the two rounds
            acc = [mktile([B, EC], f32, name=f"acc{i}") for i in range(T)]
            accv = mktile([B, EC], f32, name="accv")
            acca1 = mktile([B, EC], bf16, name="acca1")
            acca2 = mktile([B, EC], bf16, name="acca2")
            sacc = mktile([B, EC, NACT], bf16, name="sacc")
            gat = [mktile([128, NCORES * B], fp8, name=f"gat{i}")
                   for i in range(T - 1)]
            ag_sh = [mktile([NCORES * 128, B], fp8, space="DRAM",
                            addr_space="Shared", name=f"ag{i}")
                     for i in range(T - 1)]

            def mk_lhs(i):
                srct = m0t if i == 0 else gat[i - 1]
                return [srct[:, dc * 2 * B:(dc + 1) * 2 * B]
                        .rearrange("p (i b) -> p i b", i=2) for dc in range(DCH)]

            def emit_mm_round(i, lhs, groups, use_psl=False):
                # use_psl: steps 1-3 run 8-wide rounds by borrowing the two
                # LSTM-pool banks (the LSTM is long finished by then)
                pts = {}
                for jx, rg in enumerate(groups):
                    pool = psl if (use_psl and jx >= 6) else ps
                    pt_ = pool.tile([B, 512], f32, name=f"pm{i}_{rg}", tag="bank")
                    for dc in range(DCH):
                        off = (rg * DCH + dc) * 1024
                        nc.tensor.matmul(
                            pt_[:], lhs[dc],
                            kbt[:, off:off + 1024]
                            .rearrange("p (i n) -> p i n", i=2),
                            start=(dc == 0), stop=(dc == DCH - 1),
                            perf_mode=PM.DoubleRow)
                    pts[rg] = pt_
                return pts

            def emit_consume(i, pts, state, dve_only=False, dst=None):
                # dve_only: step 0 — Act is saturated by the LSTM there, so
                # the whole r-chain runs on DVE (which is idle at that point)
                tgt = dst if dst is not None else accv
                for rg in sorted(pts):
                    for rl in range(4):
                        r = rg * 4 + rl
                        src = pts[rg][:, rl * 128:(rl + 1) * 128]
                        if dve_only or r_on_dve(rg, rl):
                            nc.vector.scalar_tensor_tensor(
                                tgt[:], src, h2[i][:, r:r + 1], tgt[:],
                                ALU.mult,
                                ALU.bypass if state["first"] else ALU.add)
                            state["first"] = False
                        else:
                            k = act_k[(rg, rl)]
                            nc.scalar.mul(sacc[:, :, k:k + 1].squeeze(),
                                          src, h2[i][:, r:r + 1])

            def emit_premix(i):
                # weighted sum of the OLD mem slices for step i's outgoing
                # prev chunk — independent of acc[i], so it runs during the
                # step's matmuls; the tail then adds just the acc[i] term
                mixp = mktile([B, EC], f32, name=f"mixp{i}")
                for k in range(i + 1):
                    msl = m0s[:] if k == 0 else acc[k - 1][:]
                    nc.vector.scalar_tensor_tensor(
                        mixp[:], msl, att[i + 1][:, k:k + 1], mixp[:],
                        ALU.mult, ALU.bypass if k == 0 else ALU.add)
                return mixp

            def emit_tail(i, mixp=None):
                if i < T - 1:
                    # mix next step's prev chunk (own entity block) from local
                    # mem history (fp8 out — this IS the intended prev
                    # quantization), transpose, ship, all-gather
                    mixc = mktile([B, EC], bf16, name=f"mix{i}")
                    if mixp is None:
                        mixp = emit_premix(i)
                    nc.vector.scalar_tensor_tensor(
                        mixc[:], acc[i][:], att[i + 1][:, i + 1:i + 2], mixp[:],
                        ALU.mult, ALU.add)
                    ptm = ps.tile([128, B], bf16, name=f"ptm{i}", tag="bank")
                    nc.tensor.transpose(ptm[:], mixc[:], identb[:])
                    oc8 = mktile([128, B], fp8, name=f"oc8_{i}")
                    nc.scalar.copy(oc8[:], ptm[:])
                    bn = dram.tile([128, B], fp8, name=f"bn{i}")
                    nc.sync.dma_start(bn[:], oc8[:])
                    nc.gpsimd.collective_compute(
                        "AllGather", ALU.bypass,
                        replica_groups=[list(range(NCORES))],
                        ins=[bn.opt()], outs=[ag_sh[i].opt()])
                    nc.sync.dma_start(
                        gat[i][:],
                        ag_sh[i][:].rearrange("(k p) b -> p k b", k=NCORES))
                else:
                    # final: partial dot with tail slice, summed on host
                    fscr = mktile([B, EC], f32, name="fin_scr")
                    zcol = mktile([B, 1], f32, name="zc_sb")
                    nc.vector.scalar_tensor_tensor(
                        fscr[:], acc[i][:], 1.0, tailb[:], ALU.bypass, ALU.mult,
                        accum_out=zcol[:])
                    nc.sync.dma_start(out_d[:], zcol[:])

            # Column-major LSTM (valid order: cell (l,t) needs (l,t-1) and
            # (l-1,t)) interleaved with step 0's rounds: the t=0 column gives
            # h2[0] after ~4 cells, so step-0 accumulation streams behind the
            # kbt DMA instead of behind the whole LSTM. Step 0 consumes on
            # DVE only — Act is saturated by the LSTM at that point.
            lhs0 = mk_lhs(0)
            st0 = {"first": True}
            pts00 = emit_mm_round(0, lhs0, ROUNDS[0])
            for l in range(T):
                emit_cell(l, 0)
            emit_head(0)
            emit_consume(0, pts00, st0, dve_only=True, dst=acc[0])
            for l in range(T):
                emit_cell(l, 1)
            emit_head(1)
            pts01 = emit_mm_round(0, lhs0, ROUNDS[1])
            emit_consume(0, pts01, st0, dve_only=True, dst=acc[0])
            for l in range(T):
                emit_cell(l, 2)
            emit_head(2)
            pts02 = emit_mm_round(0, lhs0, ROUNDS[2])
            emit_consume(0, pts02, st0, dve_only=True, dst=acc[0])
            emit_tail(0)
            for l in range(T):
                emit_cell(l, 3)
            emit_head(3)

            ROUNDS8 = [list(range(0, 8)), list(range(8, 16))]
            for i in range(1, T):
                lhs = mk_lhs(i)
                state = {"first": True}
                mixp = emit_premix(i) if i < T - 1 else None
                for ri, rnd in enumerate(ROUNDS8):
                    pts = emit_mm_round(i, lhs, rnd, use_psl=True)
                    emit_consume(i, pts, state)
                    # reduce this round's Act-scaled slab now; round 1's
                    # reduce overlaps round 2's matmuls and scales
                    dst = acca1 if ri == 0 else acca2
                    with nc.allow_low_precision("18-term bf16 reduce; slack"):
                        nc.vector.tensor_reduce(
                            dst[:], sacc[:, :, ri * NACTH:(ri + 1) * NACTH],
                            AX.X, ALU.add)
                nc.vector.tensor_add(acc[i][:], accv[:], acca1[:])
                nc.vector.tensor_add(acc[i][:], acc[i][:], acca2[:])
                emit_tail(i, mixp)
            for f in reversed(_frees):
                f()
    # CoreV3 allows at most 1 sync wait per instruction (2 on EventSemaphore);
    # reuse the Bacc rust passes to split overloaded waits.
    from concourse.bacc import _bass_rust
    _bass_rust.move_matmul_waits_to_ldweights(nc.m)
    _cap_pe_waits(nc, cap_sem)
    return nc


_CAP_SKIP = ("InstDrain", "InstEventSemaphore",
             "InstCollectiveCompute", "InstUnconditionalBranch", "InstCall")


def _cap_pe_waits(nc, cap_sem):
    # CoreV3 engine command structs hold only 1 sync wait. PE/Activation get
    # excess waits moved onto same-engine EventSemaphore insts. DVE (and any
    # other engine) cannot carry event sems through lower_dve, so their waits
    # are absorbed by SP-engine event sems (SP is idle here) that each inc a
    # shared counter; the instruction then waits counter >= running total.
    act_eng = nc.sync.engine
    total = 0
    for fn in nc.m.functions:
        for bb in fn.blocks:
            snapshot = list(bb.instructions)
            edits = []
            for k, ins in enumerate(snapshot):
                if ins.__class__.__name__ in _CAP_SKIP:
                    continue
                eng = str(getattr(ins, "engine", "")).split(".")[-1]
                si = ins.sync_info
                if si is None or len(si.on_wait) <= 1:
                    continue
                waits = list(si.on_wait)
                # An in-order engine that issues no DMAs trivially satisfies
                # waits on its own tick sem — drop them instead of absorbing.
                if eng in ("DVE", "Activation", "PE"):
                    kept = [w for w in waits
                            if not (w.sync_type == "semaphore" and
                                    str(getattr(w, "ant_name", "")
                                        ).split("_")[0] == eng)]
                    if kept:
                        waits = kept
                    else:
                        waits = waits[-1:]
                    if len(waits) <= 1:
                        ins.sync_info = mybir.SyncInfo(
                            on_wait=waits, on_update=list(si.on_update))
                        continue
                evs = []
                if eng in ("PE", "Activation"):
                    ins.sync_info = mybir.SyncInfo(
                        on_wait=[waits[-1]], on_update=list(si.on_update))
                    for w in waits[:-1]:
                        ev = mybir.InstEventSemaphore(
                            name=nc.get_next_instruction_name())
                        ev.engine = ins.engine
                        ev.sync_info = mybir.SyncInfo(on_wait=[w], on_update=[])
                        nc.register_instruction(ev)
                        evs.append(ev)
                else:
                    for w in waits:
                        ev = mybir.InstEventSemaphore(
                            name=nc.get_next_instruction_name())
                        ev.engine = act_eng
                        ev.sync_info = mybir.SyncInfo(
                            on_wait=[w],
                            on_update=[mybir.SyncUpdate(
                                sync_type='semaphore', id=cap_sem.num,
                                ant_name=cap_sem.name,
                                update_mode='sem-inc', update_value=1)])
                        nc.register_instruction(ev)
                        evs.append(ev)
                        total += 1
                    ins.sync_info = mybir.SyncInfo(
                        on_wait=[mybir.SyncWait(
                            sync_type='semaphore', id=cap_sem.num,
                            ant_name=cap_sem.name,
                            wait_mode='sem-ge-imm', wait_value=total)],
                        on_update=list(si.on_update))
                # never split a Ldweights/Matmult pair
                kk = k
                while kk > 0 and snapshot[kk - 1].__class__.__name__ == "InstLdweights":
                    kk -= 1
                edits.append((kk, evs))
            edits.sort(key=lambda e: e[0])  # stable: equal kk keeps discovery order
            for k, evs in reversed(edits):
                for ev in reversed(evs):
                    bb.instructions.insert(k, ev)


def _prep_inputs(inputs):
    x = np.asarray(inputs["x"], np.float32)
    kb = np.asarray(inputs["kb"], np.float32)
    Wih0 = np.asarray(inputs["Wih0"], np.float32)
    Whh0 = np.asarray(inputs["Whh0"], np.float32)
    bih0 = np.asarray(inputs["bih0"], np.float32)
    bhh0 = np.asarray(inputs["bhh0"], np.float32)
    Wih = np.asarray(inputs["Wih"], np.float32)
    Whh = np.asarray(inputs["Whh"], np.float32)
    bih = np.asarray(inputs["bih"], np.float32)
    bhh = np.asarray(inputs["bhh"], np.float32)

    # kbt[c][p, rg*DCH*1024 + dc*1024 + i*512 + rl*128 + ep]
    #   = kb[rg*4+rl, c*128+ep, dc*256+i*128+p] * SC   (fp8)
    kb8 = (kb * SC).astype(ml_dtypes.float8_e4m3)
    kb7 = kb8.reshape(16, 4, NCORES, 128, DCH, 2, 128)  # rg rl c ep dc i p
    kbt_all = np.ascontiguousarray(
        kb7.transpose(2, 6, 0, 4, 5, 1, 3)).reshape(NCORES, 128, 16 * DCH * 1024)

    mem0 = np.ascontiguousarray(x[:, R:R + E])
    # m0t[p, dc*2B + i*B + b] = mem0[b, dc*256+i*128+p]  (fp8)
    m0t = np.ascontiguousarray(
        mem0.T.reshape(DCH, 2, 128, B).transpose(2, 0, 1, 3)
    ).reshape(128, DCH * 2 * B).astype(ml_dtypes.float8_e4m3)
    m0s = [np.ascontiguousarray(mem0[:, c * EC:(c + 1) * EC])
           for c in range(NCORES)]
    tails = [np.ascontiguousarray(x[:, R + E + c * EC:R + E + (c + 1) * EC])
             for c in range(NCORES)]

    # xtp[p, q*B + j] = x[j, q*128 + p] (zero-padded input dim)
    xT = np.zeros((INP, B), np.float32)
    xT[:IN] = x.T
    xtp = np.ascontiguousarray(
        xT.reshape(NCH, 128, B).transpose(1, 0, 2)).reshape(128, NCH * B)
    xtp = xtp.astype(ml_dtypes.bfloat16)

    # w0[p, q*G4 + g] = Wih0[g, q*128 + p] (zero-padded input dim)
    w0T = np.zeros((INP, G4), np.float32)
    w0T[:IN] = Wih0.T
    w0 = np.ascontiguousarray(
        w0T.reshape(NCH, 128, G4).transpose(1, 0, 2)).reshape(128, NCH * G4)
    w0 = w0.astype(ml_dtypes.bfloat16)

    whhT = np.concatenate([Whh0.T] + [Whh[l].T for l in range(T - 1)], axis=1)
    whhT = np.ascontiguousarray(whhT).astype(ml_dtypes.bfloat16)
    wihT = np.ascontiguousarray(
        np.concatenate([Wih[l].T for l in range(T - 1)], axis=1)
    ).astype(ml_dtypes.bfloat16)
    bvec = np.stack([bih0 + bhh0] + [bih[l] + bhh[l] for l in range(T - 1)])
    biasT = np.ascontiguousarray(
        bvec.reshape(T, 2, 128).transpose(2, 0, 1)).reshape(128, T * 2)
    biasT = biasT.astype(np.float32)

    in_maps = []
    for c in range(NCORES):
        in_maps.append({
            "kbt": kbt_all[c],
            "m0t": m0t,
            "m0s": m0s[c],
            "tail": tails[c],
            "xtp": xtp,
            "w0": w0,
            "whh": whhT,
            "wih": wihT,
            "bias": biasT,
        })
    return in_maps


_CACHED = {}


def kernel(**inputs) -> np.ndarray:
    if "nc" not in _CACHED:
        _CACHED["nc"] = build_program()
    nc = _CACHED["nc"]
    in_maps = _prep_inputs(inputs)
    res = run_bass_kernel_spmd(nc, in_maps, list(range(NCORES)), trace=False)
    z = np.zeros((B, 1), np.float64)
    for c in range(NCORES):
        z += np.asarray(res.results[c]["out"], np.float64).reshape(B, 1)
    score = 1.0 / (1.0 + np.exp(np.clip(z, -700.0, 700.0)))
    return score.astype(np.float32)


if __name__ == "__main__":
    d = np.load("/tmp/inputs.npz")
    inputs = {k: d[k] for k in d.files if k != "expected"}
    expected = d["expected"]
    out = kernel(**inputs)
    denom = np.maximum(np.abs(expected), 1e-30)
    rel = np.abs(out - expected) / denom
    rel = np.where((expected == 0) & (out == 0), 0.0, rel)
    print("rel err:", float(rel.max()), "abs:", float(np.abs(out - expected).max()))
